# revision 74
# baseline (speedup 1.0000x reference)
"""Trainium2 Bass kernel for Conv2DCollapse_w_pillar (pillar scatter -> dense BEV).

Strategy ("one-hot matmul scatter"), data-parallel over batch (1 batch / core):
  - Host: dedup pillar rows per flat cell (last write wins, matching the
    reference), sort by cell, bucket into 256-cell blocks, pad each block to K
    rows.  Features are rounded to one bf16 plane (rel err ~1e-3, well under
    the 2e-2 gate).  Row layout: even-block rows at partitions 0:K, zero gap
    K:64, odd-block rows at 64:64+K — 32-aligned partition bases for every
    on-device engine op.  Chunks 0-2 ship host-zero-stuffed block-diagonal
    ([64+K, pairs, 128], fully contiguous 16 KB DMA runs; plants the zero
    quadrants in all three rotating lhs buffers); chunks 3-7 ship PACKED
    (64 cols, half the bytes) into staging tiles, and DVE expands them into
    the block-diagonal quadrants at its 4x SBUF-copy rate (~0.7 us per
    quarter chunk), interleaved between one-hot builds.
  - Device: per pair of blocks, build one one-hot matrix
    oh[i, j] = (cell_id[i] == j) (DVE mostly, Pool for 1-2 per window), then
    a single bf16 matmul with the block-diagonal [64+K, 128] stationary
    scatter+transposes the pair into PSUM (128 partitions = 2 blocks x 64
    channels); the gap rows multiply all-zero one-hot rows.  PSUM is one
    8-bank tile manually rotated in 4 windows of 4 pairs; per super of 4
    windows ACT drains windows 0-2, DVE window 3 (Pool cannot read PSUM),
    all to bf16 SBUF.  Each half-super is written out as soon as drained:
    bf16 (C, ny*nx) DMAs with 512 B runs on the SP queue (host upcasts to
    f32).  Packed-chunk feature DMAs issue from Pool's SWDGE queue so they
    never park the ACT/SP sequencers.  Every output element is written
    exactly once; empty cells get 0 from all-zero one-hot columns.
"""
import sys
sys.path.insert(0, "/opt/trn_rl_repo")
import numpy as np
import ml_dtypes

BF = ml_dtypes.bfloat16
NCORES = 8
C = 64
NX = 512
NY = 512
NXY = NX * NY
BC = 256                 # cells per block
NBLK = NXY // BC         # 1024 blocks per core
NPAIR = NBLK // 2        # 512 pairs per core
CHUNK_PAIRS = 64         # pairs per feature-DMA chunk
NCHUNK = NPAIR // CHUNK_PAIRS
_cache = {}


def _build_nc(K):
    import concourse.bass as bass
    import concourse.tile as tile
    from concourse import bacc, mybir
    from contextlib import ExitStack

    dt = mybir.dt
    KO = 64 + K   # even rows 0:K, zero gap K:64, odd rows 64:64+K
    nc = bacc.Bacc("TRN2", target_bir_lowering=False, debug=False,
                   num_devices=NCORES)
    # Chunks 0-2: host-zero-stuffed block-diagonal features: row k<K =
    # even-block row k (channels in cols 0:64, zeros in 64:128), row K+k =
    # odd-block row k (zeros in 0:64, channels in 64:128).  Fully contiguous
    # DMA, 16 KB runs; also plants the zero quadrants in all three lhs
    # buffers, which later chunks never overwrite.  Chunks 3-7 ship PACKED
    # (64 cols, half the bytes); DVE expands them in-SBUF at its 4x copy
    # rate (~0.7 us per quarter chunk).
    NSTUFFED = 3
    fzs3 = nc.dram_tensor("fzs3", [KO, NSTUFFED * CHUNK_PAIRS, 128],
                          dt.bfloat16, kind="ExternalInput").ap()
    fpk = nc.dram_tensor("fpk", [KO, NPAIR, C], dt.bfloat16,
                         kind="ExternalInput").ap()
    cells_d = nc.dram_tensor("cells", [128, NPAIR], dt.float32,
                             kind="ExternalInput").ap()
    iota_d = nc.dram_tensor("iota", [128, BC], dt.bfloat16,
                            kind="ExternalInput").ap()
    out_d = nc.dram_tensor("out", [C, NXY], dt.bfloat16,
                           kind="ExternalOutput").ap()

    with tile.TileContext(nc) as tc, ExitStack() as ctx:
        const = ctx.enter_context(tc.tile_pool(name="const", bufs=1))
        lhsp = ctx.enter_context(tc.tile_pool(name="lhs", bufs=1))
        ohp = ctx.enter_context(tc.tile_pool(name="oh", bufs=32))
        outp = ctx.enter_context(tc.tile_pool(name="outb", bufs=8))
        psp = ctx.enter_context(tc.tile_pool(name="ps", bufs=1, space="PSUM"))

        iota_t = const.tile([128, BC], dt.bfloat16)
        cells_t = const.tile([128, NPAIR], dt.float32)
        sink = const.tile([128, 4], dt.float32, tag="sink", name="sink")
        nc.sync.dma_start(iota_t[:], iota_d[:])
        nc.sync.dma_start(cells_t[:], cells_d[:])
        # absorber copies: give DVE's and Pool's clocks each preamble-DMA sem
        # one at a time (hardware allows 1 embedded sync-wait per instruction)
        nc.vector.tensor_copy(sink[:, 0:1], cells_t[:, 0:1])
        nc.vector.tensor_copy(sink[:, 1:2], iota_t[:, 0:1])
        nc.gpsimd.tensor_copy(sink[:, 2:3], cells_t[:, 0:1])
        nc.gpsimd.tensor_copy(sink[:, 3:4], iota_t[:, 0:1])

        # 3 rotating stationary tiles holding the host-stuffed block-diagonal
        # layout: [K2 partitions, 64 pairs x 128 channel-cols].  3 buffers let
        # the first three chunks' feature DMAs all issue at the preamble
        # (no WAR wait), keeping the DMA engines packed during pipeline fill.
        NBUF = 3
        NSTG = 2
        lhs = [lhsp.tile([KO, CHUNK_PAIRS * 128], dt.bfloat16,
                         tag=f"lhs{b}", name=f"lhs{b}") for b in range(NBUF)]
        stg = [lhsp.tile([KO, CHUNK_PAIRS * C], dt.bfloat16,
                         tag=f"stg{b}", name=f"stg{b}") for b in range(NSTG)]

        def issue_in_dma(c, eng=None):
            """DMA chunk c's packed features into stg[c % NSTG] (c >= 3).
            Mid-stream issues ride Pool's SWDGE path: Pool has engine slack
            and the issue never parks the ACT or SP sequencers."""
            if eng is None:
                eng = nc.gpsimd
            p0 = c * CHUNK_PAIRS
            eng.dma_start(stg[c % NSTG][0:K, :],
                          fpk[0:K, p0:p0 + CHUNK_PAIRS, :])
            eng.dma_start(stg[c % NSTG][64:KO, :],
                          fpk[64:KO, p0:p0 + CHUNK_PAIRS, :])

        def stuff_sub(c, j):
            """DVE: expand quarter j of chunk c's packed rows into
            lhs[c % NBUF]'s block-diagonal data quadrants (zero quadrants
            persist from the chunk-0/1/2 stuffed DMAs).  j: 0=even pairs
            0:32, 1=odd 0:32, 2=even 32:64, 3=odd 32:64 — earliest pairs
            complete first in both halves."""
            t, s = lhs[c % NBUF], stg[c % NSTG]
            dst = t[0:KO, :].rearrange("k (p f) -> k p f", f=128)
            src = s[0:KO, :].rearrange("k (p f) -> k p f", f=C)
            hh = CHUNK_PAIRS // 2
            ps = slice(0, hh) if j < 2 else slice(hh, CHUNK_PAIRS)
            if j % 2 == 0:
                nc.vector.tensor_copy(dst[0:K, ps, 0:C], src[0:K, ps, :])
            else:
                nc.vector.tensor_copy(dst[64:KO, ps, C:128], src[64:KO, ps, :])

        # chunk 0 in two halves so the first matmuls start after half lands
        q4 = CHUNK_PAIRS // 4
        for qi in range(4):
            nc.sync.dma_start(
                lhs[0][0:KO, qi * q4 * 128:(qi + 1) * q4 * 128],
                fzs3[:, qi * q4:(qi + 1) * q4, :])
        for cc in (1, 2):
            p0 = cc * CHUNK_PAIRS
            nc.sync.dma_start(lhs[cc][0:KO, :],
                              fzs3[:, p0:p0 + CHUNK_PAIRS, :])
        issue_in_dma(3, eng=nc.sync)   # first packed chunk stages at preamble

        # PSUM: one 8-bank tile, manually rotated in 4 windows of QP pairs.
        QP = 4                   # pairs per PSUM window (1024 f32 = 2 banks)
        WINS = 4                 # rotation depth
        SUP = 4                  # windows per outb super-iteration (16 pairs)
        ps_big = psp.tile([128, WINS * QP * BC], dt.float32)

        outb = None
        for c in range(NCHUNK):
            p0 = c * CHUNK_PAIRS
            t = lhs[c % NBUF]
            stuffing = NSTUFFED - 1 <= c < NCHUNK - 1   # expanding chunk c+1
            for qg in range(CHUNK_PAIRS // QP):      # 16 windows per chunk
                if qg % SUP == 0:
                    outb = outp.tile([128, SUP * QP * BC], dt.bfloat16)
                # packed feature DMA two chunks ahead on Pool's queue (its
                # staging-buffer WAR is two stuffings back, long resolved)
                if qg == 0 and NSTUFFED + 1 <= c + 2 < NCHUNK:
                    issue_in_dma(c + 2)
                # DVE expands the next chunk's staged rows in 4 quarter
                # copies spread across windows 1/3/5/7 (lhs WAR: chunk c-2's
                # ldweights, three buffers back — long resolved)
                if stuffing and qg in (1, 3, 5, 7):
                    stuff_sub(c + 1, (qg - 1) // 2)
                w = qg % WINS
                pw = ps_big[:, w * QP * BC:(w + 1) * QP * BC]
                for i in range(QP):
                    p = p0 + qg * QP + i
                    oh = ohp.tile([KO, BC], dt.bfloat16)
                    # every 4th one-hot on Pool, plus a 2nd one per odd
                    # window while DVE is busy expanding a packed chunk
                    veng = (nc.gpsimd
                            if (i == QP - 1
                                or (stuffing and i == 1 and qg % 2 == 1))
                            else nc.vector)
                    veng.tensor_scalar(
                        oh[:], iota_t[0:KO, :], cells_t[0:KO, p:p + 1], None,
                        mybir.AluOpType.is_equal)
                    sl = qg * QP + i
                    nc.tensor.matmul(
                        pw[:, i * BC:(i + 1) * BC],
                        t[0:KO, sl * 128:(sl + 1) * 128],
                        oh[:], start=True, stop=True)
                # drains: ACT takes windows 0-2, DVE window 3 (Pool cannot
                # read PSUM)
                qq = (qg % SUP) * QP * BC
                if qg % SUP == 3:
                    nc.vector.tensor_copy(outb[:, qq:qq + QP * BC], pw)
                else:
                    nc.scalar.copy(outb[:, qq:qq + QP * BC], pw)
                if qg % SUP == 1 or qg % SUP == SUP - 1:
                    # write out each half-super as soon as its two windows
                    # are drained (the second half's drains include DVE's
                    # window 3): smaller DMAs issue ~1.5 us earlier
                    hb = 0 if qg % SUP == 1 else 1
                    nwin = SUP // 2
                    base = (p0 + (qg - 1) * QP) * 2 * BC
                    nblock = nwin * QP * 2
                    dst4 = out_d[:, base:base + nblock * BC].rearrange(
                        "c (p q r) -> c p q r", p=nwin * QP, q=2, r=BC)
                    ho = hb * nwin * QP * BC
                    src_e = outb[0:C, ho:ho + nwin * QP * BC].rearrange(
                        "c (p r) -> c p r", r=BC)
                    src_o = outb[C:128, ho:ho + nwin * QP * BC].rearrange(
                        "c (p r) -> c p r", r=BC)
                    nc.sync.dma_start(dst4[:, :, 0, :], src_e)
                    nc.sync.dma_start(dst4[:, :, 1, :], src_o)
    nc.compile()
    return nc


def _prep_core(pf, cell, src, K):
    """pf: (Nb, C) f32 features for this batch (deduped, sorted by cell);
    cell: (Nb,) int cell ids; src unused (rows already gathered)."""
    n = len(cell)
    block = cell // BC
    local = (cell % BC).astype(np.float32)
    starts = np.searchsorted(block, np.arange(NBLK))
    k = np.arange(n) - starts[block]
    assert k.max(initial=0) < K
    pair = block // 2
    parity = block % 2

    bf = pf.astype(BF)

    ev = parity == 0
    od = ~ev
    # packed plane [64+K, NPAIR, C]: even rows 0:K, zero gap K:64 (32-aligned
    # partition bases for the on-device expansion copies), odd rows 64:64+K
    KO = 64 + K
    fpk = np.zeros((KO, NPAIR, C), dtype=BF)
    fpk[k[ev], pair[ev], :] = bf[ev]
    fpk[64 + k[od], pair[od], :] = bf[od]
    # chunks 0-2 pre-stuffed block-diagonal [64+K, 3*CHUNK_PAIRS, 128]
    P3 = 3 * CHUNK_PAIRS
    fzs3 = np.zeros((KO, P3, 128), dtype=BF)
    fzs3[0:K, :, 0:C] = fpk[0:K, 0:P3, :]
    fzs3[64:KO, :, C:128] = fpk[64:KO, 0:P3, :]
    cells = np.full((128, NPAIR), -1.0, np.float32)
    cells[k[ev], pair[ev]] = local[ev]
    cells[64 + k[od], pair[od]] = local[od]
    m = {"fpk": fpk, "fzs3": fzs3}
    m["cells"] = cells
    m["iota"] = np.broadcast_to(
        np.arange(BC, dtype=np.float32), (128, BC)).astype(BF).copy()
    return m


def kernel(pillar_features, coords, batch_size, nx, ny, num_bev_features,
           **_ignored):
    from concourse import bass_utils

    pf = np.ascontiguousarray(np.asarray(pillar_features, dtype=np.float32))
    co = np.asarray(coords).astype(np.int64)
    B = int(batch_size)
    nx_i, ny_i, C_i = int(nx), int(ny), int(num_bev_features)
    assert (B, nx_i, ny_i, C_i) == (NCORES, NX, NY, C), "hardcoded shape mismatch"

    key = co[:, 0] * NXY + co[:, 1] + co[:, 2] * NX + co[:, 3]
    # dedup, last occurrence wins (matches reference .at[].set semantics)
    n = len(key)
    u, first_rev = np.unique(key[::-1], return_index=True)
    src = n - 1 - first_rev           # original row index that survives
    # u is sorted by (batch, cell)
    batch = (u // NXY).astype(np.int64)
    cell = (u % NXY).astype(np.int64)
    bstart = np.searchsorted(batch, np.arange(NCORES + 1))

    # K: max rows in any 256-cell block, rounded up (shared by all cores)
    blk_global = u // BC
    Kmax = int(np.max(np.bincount(blk_global, minlength=1))) if len(u) else 1
    # exact K (no rounding): every input row count scales with K, and no
    # partition-alignment rule involves K (only the 0/32/64 bases do)
    K = max(8, Kmax)
    assert K <= 64, f"block occupancy {Kmax} too high for pair kernel"

    if K not in _cache:
        _cache[K] = _build_nc(K)
    nc = _cache[K]

    in_maps = []
    for b in range(NCORES):
        lo_i, hi_i = bstart[b], bstart[b + 1]
        in_maps.append(_prep_core(pf[src[lo_i:hi_i]], cell[lo_i:hi_i],
                                  None, K))

    import os
    trace = bool(os.environ.get("BASS_TRACE"))
    res = bass_utils.run_bass_kernel_spmd(
        nc, in_maps, core_ids=list(range(NCORES)), trace=trace)
    kernel._last_results = res

    out = np.empty((NCORES, C, NY, NX), dtype=np.float32)
    for b in range(NCORES):
        out[b] = res.results[b]["out"].astype(np.float32).reshape(C, NY, NX)
    return out
